# revision 4
# baseline (speedup 1.0000x reference)
"""Single-head causal attention (B=16, S=2048, d_model=384, d_q=64) on 8 trn2 cores.

Sharding: data-parallel over batch — 2 batches per core.

Per-core kernel design (all fp32):
  - x [2, S, D] is loaded naturally and transposed on the PE (via identity
    matmuls) into xT [D, S] per batch.
  - Projections: Q^T, K^T [64, S] = (W^T stationary? no) lhsT=W chunk [128d, 64e],
    rhs = xT [128d, s] -> out [e, s].  V is computed *naturally* [s, 64] with
    lhsT = xT chunk (stationary) and rhs = Wv chunk, accumulating over d-chunks,
    then augmented with a ones column -> V_aug [128, 65] per key block.
  - Attention is computed in *transposed* score layout: scoresT[k, q] =
    matmul(lhsT=K^T block [64, 128], rhs=Q^T [64, q]).  Softmax reduction over
    keys is then a matmul reduction: out^T_aug[e', q] accumulates
    V_aug^T @ P over key blocks, where row 64 (the ones column) produces the
    softmax denominator for free.  exp() runs on the scalar engine straight
    from PSUM with the 1/sqrt(d_q) scale folded in; no max-subtraction is
    needed (scores are O(+-10), exp stays well inside fp32 range).
  - Causal masking: score matmuls only cover q >= 128*i (block diagonal
    onward); the diagonal 128x128 block is masked after exp by zeroing the
    strictly-lower-triangle (q < k) with gpsimd.affine_select.
  - Epilogue: out^T_aug [65, q] is PE-transposed back to [q, 65]; the output
    is out[:, :64] * (1 / out[:, 64]) per row, then DMA'd out.
"""

import numpy as np

B, S, D, E = 16, 2048, 384, 64
N_CORES = 8
BPC = B // N_CORES  # batches per core
NB = S // 128  # 16 key/query blocks of 128
NCH = S // 512  # 4 psum-bank-sized chunks
SCALE = 1.0 / 8.0  # 1/sqrt(d_q)

_cache = {}


def _split_multi_waits(nc, max_waits=1):
    """Walrus codegen on this image rejects instructions carrying more than
    one sync wait (setupSyncWait: 'Too many sync wait commands').  Engines
    execute their queue in order, so excess waits can be moved onto NOP
    instructions inserted immediately before the owning instruction."""
    import concourse.mybir as mybir

    k = 0
    for f in nc.m.functions:
        for bb in f.blocks:
            insts = bb.instructions
            out = []
            changed = False
            for ins in insts:
                si = getattr(ins, "sync_info", None)
                waits = list(si.on_wait) if si is not None else []
                if len(waits) > max_waits:
                    changed = True
                    for extra in waits[:-max_waits]:
                        nop = mybir.InstNoOp(
                            name=f"wsplit-{k}", ins=[], outs=[]
                        )
                        k += 1
                        nop.engine = ins.engine
                        nop.sync_info = mybir.SyncInfo(
                            on_wait=[extra], on_update=[]
                        )
                        out.append(nop)
                    ins.sync_info = mybir.SyncInfo(
                        on_wait=waits[-max_waits:],
                        on_update=list(si.on_update),
                    )
                out.append(ins)
            if changed:
                bb.instructions = out


def _install_patches():
    """Register the NTFF profile hook so trace=True works under axon."""
    import sys
    import types

    if "antenv.axon_hooks" not in sys.modules:
        mod = types.ModuleType("antenv.axon_hooks")
        state = {"hook": None}
        mod.set_axon_ntff_profile_hook = lambda h: state.__setitem__("hook", h)
        mod.get_axon_ntff_profile_hook = lambda: state["hook"]
        sys.modules["antenv.axon_hooks"] = mod
        try:
            import antenv

            antenv.axon_hooks = mod
            if "/root/.axon_site" not in sys.path:
                sys.path.insert(0, "/root/.axon_site")
            from trn_agent_boot.trn_boot import _ntff_profile_via_ctypes

            mod.set_axon_ntff_profile_hook(
                _ntff_profile_via_ctypes("/opt/axon/libaxon_pjrt.so")
            )
        except Exception:
            pass
    import concourse.bass_utils as bu

    bu.upload_artifacts = lambda tmpdir: tmpdir


def _build_nc():
    import concourse.bass as bass
    import concourse.mybir as mybir
    from concourse.bass import ts
    from concourse.masks import make_identity
    from concourse.tile import TileContext

    f32 = mybir.dt.float32
    Exp = mybir.ActivationFunctionType.Exp

    nc = bass.Bass()
    x_d = nc.dram_tensor("x", [BPC, S, D], f32, kind="ExternalInput")
    wq_d = nc.dram_tensor("wq", [D, E], f32, kind="ExternalInput")
    wk_d = nc.dram_tensor("wk", [D, E], f32, kind="ExternalInput")
    wv_d = nc.dram_tensor("wv", [D, E], f32, kind="ExternalInput")
    out_d = nc.dram_tensor("out", [BPC, S, E], f32, kind="ExternalOutput")

    with TileContext(nc) as tc:
        with (
            tc.tile_pool(name="consts", bufs=1) as cpool,
            tc.tile_pool(name="xnat", bufs=3) as xpool,
            tc.tile_pool(name="xt", bufs=6) as xtpool,
            tc.tile_pool(name="qt", bufs=2) as qtpool,
            tc.tile_pool(name="kt", bufs=2) as ktpool,
            tc.tile_pool(name="vaug", bufs=2 * NB) as vpool,
            tc.tile_pool(name="pt", bufs=3) as ptpool,
            tc.tile_pool(name="ott", bufs=2) as otpool,
            tc.tile_pool(name="otile", bufs=4) as opool,
            tc.tile_pool(name="ps", bufs=4, space="PSUM") as pspool,
            tc.tile_pool(name="acc", bufs=1, space="PSUM") as accpool,
        ):
            ident = cpool.tile([128, 128], f32, tag="ident")
            make_identity(nc, ident[:])

            # weights: chunk c of W lives at cols [c*E, (c+1)*E)
            w_sbs = []
            for name, wd in (("wq", wq_d), ("wk", wk_d), ("wv", wv_d)):
                w_sb = cpool.tile([128, 3 * E], f32, tag=name)
                for c in range(3):
                    nc.sync.dma_start(w_sb[:, ts(c, E)], wd[ts(c, 128), :])
                w_sbs.append(w_sb)
            wq_sb, wk_sb, wv_sb = w_sbs

            for b in range(BPC):
                # ---- x transpose: xT[c] [128d, S] ----
                xt = [
                    xtpool.tile([128, S], f32, tag="xt", name=f"xt_b{b}_c{c}")
                    for c in range(3)
                ]
                for t in range(NB):
                    xn = xpool.tile([128, D], f32, tag="xn")
                    nc.sync.dma_start(xn[:], x_d[b, ts(t, 128), :])
                    for c in range(3):
                        pxt = pspool.tile([128, 512], f32, tag="ps")
                        nc.tensor.transpose(
                            pxt[:, :128], xn[:, ts(c, 128)], ident[:]
                        )
                        nc.vector.tensor_copy(xt[c][:, ts(t, 128)], pxt[:, :128])

                # ---- projections: QT, KT [64, S] ----
                qt = qtpool.tile([64, S], f32, tag="qt")
                kt = ktpool.tile([64, S], f32, tag="kt")
                for w_sb, dst in ((wq_sb, qt), (wk_sb, kt)):
                    for n in range(NCH):
                        pq = pspool.tile([128, 512], f32, tag="ps")
                        for c in range(3):
                            nc.tensor.matmul(
                                pq[:64, :],
                                w_sb[:, ts(c, E)],
                                xt[c][:, ts(n, 512)],
                                start=(c == 0),
                                stop=(c == 2),
                            )
                        nc.vector.tensor_copy(dst[:, ts(n, 512)], pq[:64, :])

                # ---- V natural + ones column: V_aug [128, 65] per key block ----
                vts = []
                for k in range(NB):
                    pv = pspool.tile([128, 512], f32, tag="ps")
                    for c in range(3):
                        nc.tensor.matmul(
                            pv[:, :E],
                            xt[c][:, ts(k, 128)],
                            wv_sb[:, ts(c, E)],
                            start=(c == 0),
                            stop=(c == 2),
                        )
                    va = vpool.tile([128, E + 1], f32, tag="va")
                    nc.vector.tensor_copy(va[:, 0:E], pv[:, :E])
                    nc.gpsimd.memset(va[:, E : E + 1], 1.0)
                    vts.append(va)

                # ---- attention: key-block outer loop ----
                acc = accpool.tile([E + 1, S], f32, tag="acc")
                for i in range(NB):
                    qlo = 128 * i
                    for c in range(i // 4, NCH):
                        lo = max(512 * c, qlo)
                        hi = 512 * (c + 1)
                        w = hi - lo
                        pscr = pspool.tile([128, 512], f32, tag="ps")
                        nc.tensor.matmul(
                            pscr[:, :w],
                            kt[:, ts(i, 128)],
                            qt[:, lo:hi],
                            start=True,
                            stop=True,
                        )
                        pt = ptpool.tile([128, 512], f32, tag="pt")
                        nc.scalar.activation(
                            pt[:, :w], pscr[:, :w], Exp, scale=SCALE
                        )
                        if c == i // 4:
                            # zero strictly-lower triangle (q < k) of the
                            # diagonal 128-block, which is cols [0, 128)
                            nc.gpsimd.affine_select(
                                out=pt[:, 0:128],
                                in_=pt[:, 0:128],
                                compare_op=mybir.AluOpType.is_ge,
                                fill=0.0,
                                base=0,
                                pattern=[[1, 128]],
                                channel_multiplier=-1,
                            )
                        nc.tensor.matmul(
                            acc[:, lo:hi],
                            vts[i][:],
                            pt[:, :w],
                            start=(i == 0),
                            stop=(i == 4 * c + 3),
                        )

                # ---- epilogue: transpose back, normalize, store ----
                ott = otpool.tile([E + 1, S], f32, tag="ott")
                for n in range(NCH):
                    nc.vector.tensor_copy(ott[:, ts(n, 512)], acc[:, ts(n, 512)])
                for t in range(NB):
                    pe_ = pspool.tile([128, 512], f32, tag="ps")
                    nc.tensor.transpose(
                        pe_[:, : E + 1], ott[:, ts(t, 128)], ident[: E + 1, : E + 1]
                    )
                    rc = opool.tile([128, 1], f32, tag="rc")
                    nc.vector.reciprocal(rc[:], pe_[:, E : E + 1])
                    ot = opool.tile([128, E], f32, tag="ot")
                    nc.vector.tensor_scalar_mul(ot[:], pe_[:, 0:E], rc[:])
                    nc.sync.dma_start(out_d[b, ts(t, 128), :], ot[:])

    _split_multi_waits(nc)
    return nc


def _get_nc():
    if "nc" not in _cache:
        _install_patches()
        _cache["nc"] = _build_nc()
    return _cache["nc"]


def kernel(x, Wq, Wk, Wv):
    from concourse.bass_utils import run_bass_kernel_spmd

    nc = _get_nc()
    x = np.ascontiguousarray(x, dtype=np.float32)
    in_maps = [
        {
            "x": x[i * BPC : (i + 1) * BPC],
            "wq": np.asarray(Wq, dtype=np.float32),
            "wk": np.asarray(Wk, dtype=np.float32),
            "wv": np.asarray(Wv, dtype=np.float32),
        }
        for i in range(N_CORES)
    ]
    res = run_bass_kernel_spmd(nc, in_maps, list(range(N_CORES)))
    out = np.concatenate([res.results[i]["out"] for i in range(N_CORES)], axis=0)
    return out.astype(np.float32)


# revision 8
# speedup vs baseline: 1.6482x; 1.6482x over previous
"""Single-head causal attention (B=16, S=2048, d_model=384, d_q=64) on 8 trn2 cores.

Sharding: data-parallel over batch — 2 batches per core.

Per-core kernel design (all fp32):
  - x [2, S, D] is loaded naturally and transposed on the PE (via identity
    matmuls) into xT [D, S] per batch.
  - Projections: Q^T, K^T [64, S] = (W^T stationary? no) lhsT=W chunk [128d, 64e],
    rhs = xT [128d, s] -> out [e, s].  V is computed *naturally* [s, 64] with
    lhsT = xT chunk (stationary) and rhs = Wv chunk, accumulating over d-chunks,
    then augmented with a ones column -> V_aug [128, 65] per key block.
  - Attention is computed in *transposed* score layout: scoresT[k, q] =
    matmul(lhsT=K^T block [64, 128], rhs=Q^T [64, q]).  Softmax reduction over
    keys is then a matmul reduction: out^T_aug[e', q] accumulates
    V_aug^T @ P over key blocks, where row 64 (the ones column) produces the
    softmax denominator for free.  exp() runs on the scalar engine straight
    from PSUM with the 1/sqrt(d_q) scale folded in; no max-subtraction is
    needed (scores are O(+-10), exp stays well inside fp32 range).
  - Causal masking: score matmuls only cover q >= 128*i (block diagonal
    onward); the diagonal 128x128 block is masked after exp by zeroing the
    strictly-lower-triangle (q < k) with gpsimd.affine_select.
  - Epilogue: out^T_aug [65, q] is PE-transposed back to [q, 65]; the output
    is out[:, :64] * (1 / out[:, 64]) per row, then DMA'd out.
"""

import numpy as np

B, S, D, E = 16, 2048, 384, 64
N_CORES = 8
BPC = B // N_CORES  # batches per core
NB = S // 128  # 16 key/query blocks of 128
NCH = S // 512  # 4 psum-bank-sized chunks
SCALE = 1.0 / 8.0  # 1/sqrt(d_q)

_cache = {}


def _split_multi_waits(nc, max_waits=1):
    """Walrus codegen on this image rejects instructions carrying more than
    one sync wait (setupSyncWait: 'Too many sync wait commands').  Engines
    execute their queue in order, so excess waits can be moved onto NOP
    instructions inserted immediately before the owning instruction."""
    import concourse.mybir as mybir

    k = 0
    for f in nc.m.functions:
        for bb in f.blocks:
            insts = bb.instructions
            out = []
            changed = False
            for ins in insts:
                si = getattr(ins, "sync_info", None)
                waits = list(si.on_wait) if si is not None else []
                if len(waits) > max_waits:
                    changed = True
                    for extra in waits[:-max_waits]:
                        nop = mybir.InstNoOp(
                            name=f"wsplit-{k}", ins=[], outs=[]
                        )
                        k += 1
                        nop.engine = ins.engine
                        nop.sync_info = mybir.SyncInfo(
                            on_wait=[extra], on_update=[]
                        )
                        out.append(nop)
                    ins.sync_info = mybir.SyncInfo(
                        on_wait=waits[-max_waits:],
                        on_update=list(si.on_update),
                    )
                out.append(ins)
            if changed:
                bb.instructions = out


def _install_patches():
    """Register the NTFF profile hook so trace=True works under axon."""
    import sys
    import types

    if "antenv.axon_hooks" not in sys.modules:
        mod = types.ModuleType("antenv.axon_hooks")
        state = {"hook": None}
        mod.set_axon_ntff_profile_hook = lambda h: state.__setitem__("hook", h)
        mod.get_axon_ntff_profile_hook = lambda: state["hook"]
        sys.modules["antenv.axon_hooks"] = mod
        try:
            import antenv

            antenv.axon_hooks = mod
            if "/root/.axon_site" not in sys.path:
                sys.path.insert(0, "/root/.axon_site")
            from trn_agent_boot.trn_boot import _ntff_profile_via_ctypes

            mod.set_axon_ntff_profile_hook(
                _ntff_profile_via_ctypes("/opt/axon/libaxon_pjrt.so")
            )
        except Exception:
            pass
    import concourse.bass_utils as bu

    bu.upload_artifacts = lambda tmpdir: tmpdir


def _build_nc():
    import concourse.bass as bass
    import concourse.mybir as mybir
    from concourse.bass import ts
    from concourse.masks import make_identity
    from concourse.tile import TileContext

    f32 = mybir.dt.float32
    bf16 = mybir.dt.bfloat16
    Exp = mybir.ActivationFunctionType.Exp

    nc = bass.Bass()
    x_d = nc.dram_tensor("x", [BPC, S, D], f32, kind="ExternalInput")
    wq_d = nc.dram_tensor("wq", [D, E], f32, kind="ExternalInput")
    wk_d = nc.dram_tensor("wk", [D, E], f32, kind="ExternalInput")
    wv_d = nc.dram_tensor("wv", [D, E], f32, kind="ExternalInput")
    out_d = nc.dram_tensor("out", [BPC, S, E], f32, kind="ExternalOutput")

    with TileContext(nc) as tc:
        with (
            tc.tile_pool(name="consts", bufs=1) as cpool,
            tc.tile_pool(name="xnat", bufs=3) as xpool,
            tc.tile_pool(name="xt", bufs=6) as xtpool,
            tc.tile_pool(name="qt", bufs=2) as qtpool,
            tc.tile_pool(name="kt", bufs=2) as ktpool,
            tc.tile_pool(name="vaug", bufs=2 * NB) as vpool,
            tc.tile_pool(name="pt", bufs=3) as ptpool,
            tc.tile_pool(name="ott", bufs=2) as otpool,
            tc.tile_pool(name="otile", bufs=4) as opool,
            tc.tile_pool(name="ps", bufs=3, space="PSUM") as pspool,
            tc.tile_pool(name="acc", bufs=1, space="PSUM") as accpool,
        ):
            identb = cpool.tile([128, 128], bf16, tag="identb")
            make_identity(nc, identb[:])
            ident = cpool.tile([128, 128], f32, tag="ident")
            make_identity(nc, ident[:])

            # Wq/Wk packed side-by-side per d-chunk: chunk c occupies cols
            # [128c, 128c+64) = Wq, [128c+64, 128c+128) = Wk.  One [128,128]
            # stationary then projects Q^T and K^T in a single matmul stream.
            wstg = cpool.tile([128, 2 * E], f32, tag="wstg")
            wqk_sb = cpool.tile([128, 3 * 128], bf16, tag="wqk")
            wv_sb = cpool.tile([128, 3 * E], bf16, tag="wv")
            for c in range(3):
                nc.sync.dma_start(wstg[:, 0:E], wq_d[ts(c, 128), :])
                nc.sync.dma_start(wstg[:, E : 2 * E], wk_d[ts(c, 128), :])
                nc.vector.tensor_copy(
                    wqk_sb[:, 128 * c : 128 * c + 128], wstg[:]
                )
            for c in range(3):
                nc.sync.dma_start(wstg[:, 0:E], wv_d[ts(c, 128), :])
                nc.vector.tensor_copy(wv_sb[:, ts(c, E)], wstg[:, 0:E])

            for b in range(BPC):
                # ---- x transpose (bf16): xT[c] [128d, S] ----
                xt = [
                    xtpool.tile([128, S], bf16, tag="xt", name=f"xt_b{b}_c{c}")
                    for c in range(3)
                ]
                for t in range(NB):
                    xn = xpool.tile([128, D], f32, tag="xn")
                    nc.sync.dma_start(xn[:], x_d[b, ts(t, 128), :])
                    for c in range(3):
                        pxt = pspool.tile([128, 512], f32, tag="ps")
                        nc.tensor.transpose(
                            pxt[:, :128], xn[:, ts(c, 128)], ident[:]
                        )
                        nc.vector.tensor_copy(xt[c][:, ts(t, 128)], pxt[:, :128])

                # ---- projections: QT, KT [64, S] bf16 ----
                qt = qtpool.tile([64, S], bf16, tag="qt")
                kt = ktpool.tile([64, S], bf16, tag="kt")
                for n in range(NCH):
                    pq = pspool.tile([128, 512], f32, tag="ps")
                    for c in range(3):
                        nc.tensor.matmul(
                            pq[:],
                            wqk_sb[:, ts(c, 128)],
                            xt[c][:, ts(n, 512)],
                            start=(c == 0),
                            stop=(c == 2),
                        )
                    nc.vector.tensor_copy(qt[:, ts(n, 512)], pq[:64, :])
                    nc.vector.tensor_copy(kt[:, ts(n, 512)], pq[64:128, :])

                # ---- V natural + ones column: V_aug [128, 65] per key block ----
                vts = []
                for k in range(NB):
                    pv = pspool.tile([128, 512], f32, tag="ps")
                    for c in range(3):
                        nc.tensor.matmul(
                            pv[:, :E],
                            xt[c][:, ts(k, 128)],
                            wv_sb[:, ts(c, E)],
                            start=(c == 0),
                            stop=(c == 2),
                        )
                    va = vpool.tile([128, E + 1], bf16, tag="va")
                    nc.vector.tensor_copy(va[:, 0:E], pv[:, :E])
                    nc.gpsimd.memset(va[:, E : E + 1], 1.0)
                    vts.append(va)

                # ---- attention: key-block outer loop ----
                acc = accpool.tile([E + 1, S], f32, tag="acc")
                for i in range(NB):
                    qlo = 128 * i
                    for c in range(i // 4, NCH):
                        lo = max(512 * c, qlo)
                        hi = 512 * (c + 1)
                        w = hi - lo
                        pscr = pspool.tile([128, 512], f32, tag="ps")
                        nc.tensor.matmul(
                            pscr[:, :w],
                            kt[:, ts(i, 128)],
                            qt[:, lo:hi],
                            start=True,
                            stop=True,
                        )
                        pt = ptpool.tile([128, 512], bf16, tag="pt")
                        nc.scalar.activation(
                            pt[:, :w], pscr[:, :w], Exp, scale=SCALE
                        )
                        if c == i // 4:
                            # zero strictly-lower triangle (q < k) of the
                            # diagonal 128-block, which is cols [0, 128)
                            nc.gpsimd.affine_select(
                                out=pt[:, 0:128],
                                in_=pt[:, 0:128],
                                compare_op=mybir.AluOpType.is_ge,
                                fill=0.0,
                                base=0,
                                pattern=[[1, 128]],
                                channel_multiplier=-1,
                            )
                        nc.tensor.matmul(
                            acc[:, lo:hi],
                            vts[i][:],
                            pt[:, :w],
                            start=(i == 0),
                            stop=(i == 4 * c + 3),
                        )

                # ---- epilogue: transpose back, normalize, store ----
                ott = otpool.tile([E + 1, S], f32, tag="ott")
                for n in range(NCH):
                    nc.vector.tensor_copy(ott[:, ts(n, 512)], acc[:, ts(n, 512)])
                for t in range(NB):
                    pe_ = pspool.tile([128, 512], f32, tag="ps")
                    nc.tensor.transpose(
                        pe_[:, : E + 1], ott[:, ts(t, 128)], ident[: E + 1, : E + 1]
                    )
                    rc = opool.tile([128, 1], f32, tag="rc")
                    nc.vector.reciprocal(rc[:], pe_[:, E : E + 1])
                    ot = opool.tile([128, E], f32, tag="ot")
                    nc.vector.tensor_scalar_mul(ot[:], pe_[:, 0:E], rc[:])
                    nc.sync.dma_start(out_d[b, ts(t, 128), :], ot[:])

    _split_multi_waits(nc)
    return nc


def _get_nc():
    if "nc" not in _cache:
        _install_patches()
        _cache["nc"] = _build_nc()
    return _cache["nc"]


def kernel(x, Wq, Wk, Wv):
    from concourse.bass_utils import run_bass_kernel_spmd

    nc = _get_nc()
    x = np.ascontiguousarray(x, dtype=np.float32)
    in_maps = [
        {
            "x": x[i * BPC : (i + 1) * BPC],
            "wq": np.asarray(Wq, dtype=np.float32),
            "wk": np.asarray(Wk, dtype=np.float32),
            "wv": np.asarray(Wv, dtype=np.float32),
        }
        for i in range(N_CORES)
    ]
    res = run_bass_kernel_spmd(nc, in_maps, list(range(N_CORES)))
    out = np.concatenate([res.results[i]["out"] for i in range(N_CORES)], axis=0)
    return out.astype(np.float32)


# revision 10
# speedup vs baseline: 1.9603x; 1.1894x over previous
"""Single-head causal attention (B=16, S=2048, d_model=384, d_q=64) on 8 trn2 cores.

Sharding: data-parallel over batch — 2 batches per core.

Per-core kernel design (all fp32):
  - x [2, S, D] is loaded naturally and transposed on the PE (via identity
    matmuls) into xT [D, S] per batch.
  - Projections: Q^T, K^T [64, S] = (W^T stationary? no) lhsT=W chunk [128d, 64e],
    rhs = xT [128d, s] -> out [e, s].  V is computed *naturally* [s, 64] with
    lhsT = xT chunk (stationary) and rhs = Wv chunk, accumulating over d-chunks,
    then augmented with a ones column -> V_aug [128, 65] per key block.
  - Attention is computed in *transposed* score layout: scoresT[k, q] =
    matmul(lhsT=K^T block [64, 128], rhs=Q^T [64, q]).  Softmax reduction over
    keys is then a matmul reduction: out^T_aug[e', q] accumulates
    V_aug^T @ P over key blocks, where row 64 (the ones column) produces the
    softmax denominator for free.  exp() runs on the scalar engine straight
    from PSUM with the 1/sqrt(d_q) scale folded in; no max-subtraction is
    needed (scores are O(+-10), exp stays well inside fp32 range).
  - Causal masking: score matmuls only cover q >= 128*i (block diagonal
    onward); the diagonal 128x128 block is masked after exp by zeroing the
    strictly-lower-triangle (q < k) with gpsimd.affine_select.
  - Epilogue: out^T_aug [65, q] is PE-transposed back to [q, 65]; the output
    is out[:, :64] * (1 / out[:, 64]) per row, then DMA'd out.
"""

import numpy as np

B, S, D, E = 16, 2048, 384, 64
N_CORES = 8
BPC = B // N_CORES  # batches per core
NB = S // 128  # 16 key/query blocks of 128
NCH = S // 512  # 4 psum-bank-sized chunks
SCALE = 1.0 / 8.0  # 1/sqrt(d_q)

_cache = {}


def _split_multi_waits(nc, max_waits=1):
    """Walrus codegen on this image rejects instructions carrying more than
    one sync wait (setupSyncWait: 'Too many sync wait commands').  Engines
    execute their queue in order, so excess waits can be moved onto NOP
    instructions inserted immediately before the owning instruction."""
    import concourse.mybir as mybir

    k = 0
    for f in nc.m.functions:
        for bb in f.blocks:
            insts = bb.instructions
            out = []
            changed = False
            for ins in insts:
                si = getattr(ins, "sync_info", None)
                waits = list(si.on_wait) if si is not None else []
                if len(waits) > max_waits:
                    changed = True
                    for extra in waits[:-max_waits]:
                        nop = mybir.InstNoOp(
                            name=f"wsplit-{k}", ins=[], outs=[]
                        )
                        k += 1
                        nop.engine = ins.engine
                        nop.sync_info = mybir.SyncInfo(
                            on_wait=[extra], on_update=[]
                        )
                        out.append(nop)
                    ins.sync_info = mybir.SyncInfo(
                        on_wait=waits[-max_waits:],
                        on_update=list(si.on_update),
                    )
                out.append(ins)
            if changed:
                bb.instructions = out


def _install_patches():
    """Register the NTFF profile hook so trace=True works under axon."""
    import sys
    import types

    if "antenv.axon_hooks" not in sys.modules:
        mod = types.ModuleType("antenv.axon_hooks")
        state = {"hook": None}
        mod.set_axon_ntff_profile_hook = lambda h: state.__setitem__("hook", h)
        mod.get_axon_ntff_profile_hook = lambda: state["hook"]
        sys.modules["antenv.axon_hooks"] = mod
        try:
            import antenv

            antenv.axon_hooks = mod
            if "/root/.axon_site" not in sys.path:
                sys.path.insert(0, "/root/.axon_site")
            from trn_agent_boot.trn_boot import _ntff_profile_via_ctypes

            mod.set_axon_ntff_profile_hook(
                _ntff_profile_via_ctypes("/opt/axon/libaxon_pjrt.so")
            )
        except Exception:
            pass
    import concourse.bass_utils as bu

    bu.upload_artifacts = lambda tmpdir: tmpdir


def _build_nc():
    import concourse.bass as bass
    import concourse.mybir as mybir
    from concourse.bass import ts
    from concourse.masks import make_identity
    from concourse.tile import TileContext

    f32 = mybir.dt.float32
    bf16 = mybir.dt.bfloat16
    Exp = mybir.ActivationFunctionType.Exp

    nc = bass.Bass()
    x_d = nc.dram_tensor("x", [BPC, S, D], f32, kind="ExternalInput")
    wq_d = nc.dram_tensor("wq", [D, E], f32, kind="ExternalInput")
    wk_d = nc.dram_tensor("wk", [D, E], f32, kind="ExternalInput")
    wv_d = nc.dram_tensor("wv", [D, E], f32, kind="ExternalInput")
    out_d = nc.dram_tensor("out", [BPC, S, E], f32, kind="ExternalOutput")

    with TileContext(nc) as tc:
        with (
            tc.tile_pool(name="consts", bufs=1) as cpool,
            tc.tile_pool(name="xnat", bufs=3) as xpool,
            tc.tile_pool(name="xt", bufs=2) as xtpool,
            tc.tile_pool(name="qt", bufs=2) as qtpool,
            tc.tile_pool(name="kt", bufs=2) as ktpool,
            tc.tile_pool(name="vaug", bufs=2) as vpool,
            tc.tile_pool(name="pt", bufs=3) as ptpool,
            tc.tile_pool(name="ott", bufs=2) as otpool,
            tc.tile_pool(name="otile", bufs=4) as opool,
            tc.tile_pool(name="ps", bufs=2, space="PSUM") as pspool,
            tc.tile_pool(name="pss", bufs=2, space="PSUM") as pshalf,
            tc.tile_pool(name="acc", bufs=1, space="PSUM") as accpool,
        ):
            identb = cpool.tile([128, 128], bf16, tag="identb")
            make_identity(nc, identb[:])
            ident = cpool.tile([128, 128], f32, tag="ident")
            make_identity(nc, ident[:])

            # Wq/Wk packed side-by-side per d-chunk: chunk c occupies cols
            # [128c, 128c+64) = Wq, [128c+64, 128c+128) = Wk.  One [128,128]
            # stationary then projects Q^T and K^T in a single matmul stream.
            wstg = cpool.tile([128, 2 * E], f32, tag="wstg")
            wqk_sb = cpool.tile([128, 3 * 128], bf16, tag="wqk")
            wv_sb = cpool.tile([128, 3 * E], bf16, tag="wv")
            for c in range(3):
                nc.sync.dma_start(wstg[:, 0:E], wq_d[ts(c, 128), :])
                nc.sync.dma_start(wstg[:, E : 2 * E], wk_d[ts(c, 128), :])
                nc.vector.tensor_copy(
                    wqk_sb[:, 128 * c : 128 * c + 128], wstg[:]
                )
            for c in range(3):
                nc.sync.dma_start(wstg[:, 0:E], wv_d[ts(c, 128), :])
                nc.vector.tensor_copy(wv_sb[:, ts(c, E)], wstg[:, 0:E])

            H = 1024  # attention column-panel width
            NH = S // H

            def pieces512(lo, hi):
                out = []
                a = lo
                while a < hi:
                    b_ = min((a // 512 + 1) * 512, hi)
                    out.append((a, b_))
                    a = b_
                return out

            for b in range(BPC):
                # ---- x load + bf16 cast + PE transpose (packed) ----
                # xT lives in one tile: chunk c occupies cols [c*S, (c+1)*S)
                xt_all = xtpool.tile([128, 3 * S], bf16, tag="xt", name=f"xt_{b}")
                xt3 = xt_all[:].rearrange("p (c s) -> p c s", c=3)
                for t in range(NB):
                    xn = xpool.tile([128, D], f32, tag="xn")
                    nc.sync.dma_start(xn[:], x_d[b, ts(t, 128), :])
                    xb = xpool.tile([128, D], bf16, tag="xb")
                    nc.vector.tensor_copy(xb[:], xn[:])
                    pxt = pspool.tile([128, 512], bf16, tag="ps")
                    for c in range(3):
                        nc.tensor.transpose(
                            pxt[:, 128 * c : 128 * c + 128],
                            xb[:, ts(c, 128)],
                            identb[:],
                        )
                    nc.vector.tensor_copy(
                        xt3[:, :, ts(t, 128)],
                        pxt[:, 0:D].rearrange("p (c s) -> p c s", c=3),
                    )

                def xts(c, lo, width):
                    return xt_all[:, c * S + lo : c * S + lo + width]

                # ---- projections: QT, KT [64, S] bf16 (packed QK stationary) ----
                qt = qtpool.tile([64, S], bf16, tag="qt")
                kt = ktpool.tile([64, S], bf16, tag="kt")
                for n in range(NCH):
                    pq = pspool.tile([128, 512], f32, tag="ps")
                    for c in range(3):
                        nc.tensor.matmul(
                            pq[:],
                            wqk_sb[:, ts(c, 128)],
                            xts(c, 512 * n, 512),
                            start=(c == 0),
                            stop=(c == 2),
                        )
                    nc.vector.tensor_copy(qt[:, ts(n, 512)], pq[:64, :])
                    nc.vector.tensor_copy(kt[:, ts(n, 512)], pq[64:128, :])

                # ---- V natural + ones col: va_all[:, 65k:65k+65] = [V_k | 1] ----
                va_all = vpool.tile([128, NB * (E + 1)], bf16, tag="va", name=f"va_{b}")
                va3 = va_all[:].rearrange("p (k e) -> p k e", k=NB)
                for g in range(NB // 4):
                    pv = pspool.tile([128, 512], f32, tag="ps")
                    for j in range(4):
                        k = 4 * g + j
                        for c in range(3):
                            nc.tensor.matmul(
                                pv[:, 64 * j : 64 * j + 64],
                                xts(c, 128 * k, 128),
                                wv_sb[:, ts(c, E)],
                                start=(c == 0),
                                stop=(c == 2),
                            )
                    nc.vector.tensor_copy(
                        va3[:, 4 * g : 4 * g + 4, 0:E],
                        pv[:, 0:256].rearrange("p (k e) -> p k e", k=4),
                    )
                nc.gpsimd.memset(va3[:, :, E : E + 1], 1.0)

                # ---- attention: column-panel outer, key-block inner ----
                for h in range(NH):
                    base = H * h
                    acc = accpool.tile([E + 1, H], f32, tag="acc")
                    nblk = base // 128 + 8

                    def emit_pv(i, pt, qlo):
                        for (a, b_) in pieces512(qlo, base + H):
                            nc.tensor.matmul(
                                acc[:, a - base : b_ - base],
                                va_all[:, 65 * i : 65 * i + 65],
                                pt[:, a - qlo : b_ - qlo],
                                start=(i == 0),
                                stop=(i == (b_ - 1) // 128),
                            )

                    # software-pipelined emission: PV(i) is emitted after
                    # scores(i+1)/exp(i+1) so the (in-order) PE streams the
                    # next block's scores while ACT computes exp(i).
                    pending = None
                    for i in range(nblk):
                        qlo = max(128 * i, base)
                        w = base + H - qlo
                        ps_s = pshalf.tile([128, H], f32, tag="pss")
                        for c0 in range(0, w, 512):
                            c1 = min(c0 + 512, w)
                            nc.tensor.matmul(
                                ps_s[:, c0:c1],
                                kt[:, ts(i, 128)],
                                qt[:, qlo + c0 : qlo + c1],
                                start=True,
                                stop=True,
                            )
                        pt = ptpool.tile([128, H], bf16, tag="pt")
                        nc.scalar.activation(pt[:, :w], ps_s[:, :w], Exp, scale=SCALE)
                        if qlo == 128 * i:
                            # zero strictly-lower triangle (q < k) of diag block
                            nc.gpsimd.affine_select(
                                out=pt[:, 0:128],
                                in_=pt[:, 0:128],
                                compare_op=mybir.AluOpType.is_ge,
                                fill=0.0,
                                base=0,
                                pattern=[[1, 128]],
                                channel_multiplier=-1,
                            )
                        if pending is not None:
                            emit_pv(*pending)
                        pending = (i, pt, qlo)
                    emit_pv(*pending)

                    # ---- epilogue for this panel ----
                    ott = otpool.tile([E + 1, H], f32, tag="ott")
                    nc.vector.tensor_copy(ott[:], acc[:])
                    for tt in range(H // 128):
                        tg = (base // 128) + tt
                        pe_ = pspool.tile([128, 512], f32, tag="ps")
                        nc.tensor.transpose(
                            pe_[:, : E + 1],
                            ott[:, ts(tt, 128)],
                            ident[: E + 1, : E + 1],
                        )
                        rc = opool.tile([128, 1], f32, tag="rc")
                        nc.vector.reciprocal(rc[:], pe_[:, E : E + 1])
                        ot = opool.tile([128, E], f32, tag="ot")
                        nc.vector.tensor_scalar_mul(ot[:], pe_[:, 0:E], rc[:])
                        nc.sync.dma_start(out_d[b, ts(tg, 128), :], ot[:])

    _split_multi_waits(nc)
    return nc


def _get_nc():
    if "nc" not in _cache:
        _install_patches()
        _cache["nc"] = _build_nc()
    return _cache["nc"]


def kernel(x, Wq, Wk, Wv):
    from concourse.bass_utils import run_bass_kernel_spmd

    nc = _get_nc()
    x = np.ascontiguousarray(x, dtype=np.float32)
    in_maps = [
        {
            "x": x[i * BPC : (i + 1) * BPC],
            "wq": np.asarray(Wq, dtype=np.float32),
            "wk": np.asarray(Wk, dtype=np.float32),
            "wv": np.asarray(Wv, dtype=np.float32),
        }
        for i in range(N_CORES)
    ]
    res = run_bass_kernel_spmd(nc, in_maps, list(range(N_CORES)))
    out = np.concatenate([res.results[i]["out"] for i in range(N_CORES)], axis=0)
    return out.astype(np.float32)


# revision 11
# speedup vs baseline: 2.7868x; 1.4216x over previous
"""Single-head causal attention (B=16, S=2048, d_model=384, d_q=64) on 8 trn2 cores.

Sharding: data-parallel over batch — 2 batches per core.

Per-core kernel design (all fp32):
  - x [2, S, D] is loaded naturally and transposed on the PE (via identity
    matmuls) into xT [D, S] per batch.
  - Projections: Q^T, K^T [64, S] = (W^T stationary? no) lhsT=W chunk [128d, 64e],
    rhs = xT [128d, s] -> out [e, s].  V is computed *naturally* [s, 64] with
    lhsT = xT chunk (stationary) and rhs = Wv chunk, accumulating over d-chunks,
    then augmented with a ones column -> V_aug [128, 65] per key block.
  - Attention is computed in *transposed* score layout: scoresT[k, q] =
    matmul(lhsT=K^T block [64, 128], rhs=Q^T [64, q]).  Softmax reduction over
    keys is then a matmul reduction: out^T_aug[e', q] accumulates
    V_aug^T @ P over key blocks, where row 64 (the ones column) produces the
    softmax denominator for free.  exp() runs on the scalar engine straight
    from PSUM with the 1/sqrt(d_q) scale folded in; no max-subtraction is
    needed (scores are O(+-10), exp stays well inside fp32 range).
  - Causal masking: score matmuls only cover q >= 128*i (block diagonal
    onward); the diagonal 128x128 block is masked after exp by zeroing the
    strictly-lower-triangle (q < k) with gpsimd.affine_select.
  - Epilogue: out^T_aug [65, q] is PE-transposed back to [q, 65]; the output
    is out[:, :64] * (1 / out[:, 64]) per row, then DMA'd out.
"""

import numpy as np

B, S, D, E = 16, 2048, 384, 64
N_CORES = 8
BPC = B // N_CORES  # batches per core
NB = S // 128  # 16 key/query blocks of 128
NCH = S // 512  # 4 psum-bank-sized chunks
SCALE = 1.0 / 8.0  # 1/sqrt(d_q)

_cache = {}


def _split_multi_waits(nc, max_waits=1):
    """Walrus codegen on this image rejects instructions carrying more than
    one sync wait (setupSyncWait: 'Too many sync wait commands').  Engines
    execute their queue in order, so excess waits can be moved onto NOP
    instructions inserted immediately before the owning instruction."""
    import concourse.mybir as mybir

    k = 0
    for f in nc.m.functions:
        for bb in f.blocks:
            insts = bb.instructions
            out = []
            changed = False
            for ins in insts:
                si = getattr(ins, "sync_info", None)
                waits = list(si.on_wait) if si is not None else []
                if len(waits) > max_waits:
                    changed = True
                    for extra in waits[:-max_waits]:
                        nop = mybir.InstNoOp(
                            name=f"wsplit-{k}", ins=[], outs=[]
                        )
                        k += 1
                        nop.engine = ins.engine
                        nop.sync_info = mybir.SyncInfo(
                            on_wait=[extra], on_update=[]
                        )
                        out.append(nop)
                    ins.sync_info = mybir.SyncInfo(
                        on_wait=waits[-max_waits:],
                        on_update=list(si.on_update),
                    )
                out.append(ins)
            if changed:
                bb.instructions = out


def _install_patches():
    """Register the NTFF profile hook so trace=True works under axon."""
    import sys
    import types

    if "antenv.axon_hooks" not in sys.modules:
        mod = types.ModuleType("antenv.axon_hooks")
        state = {"hook": None}
        mod.set_axon_ntff_profile_hook = lambda h: state.__setitem__("hook", h)
        mod.get_axon_ntff_profile_hook = lambda: state["hook"]
        sys.modules["antenv.axon_hooks"] = mod
        try:
            import antenv

            antenv.axon_hooks = mod
            if "/root/.axon_site" not in sys.path:
                sys.path.insert(0, "/root/.axon_site")
            from trn_agent_boot.trn_boot import _ntff_profile_via_ctypes

            mod.set_axon_ntff_profile_hook(
                _ntff_profile_via_ctypes("/opt/axon/libaxon_pjrt.so")
            )
        except Exception:
            pass
    import concourse.bass_utils as bu

    bu.upload_artifacts = lambda tmpdir: tmpdir


def _build_nc():
    import concourse.bass as bass
    import concourse.mybir as mybir
    from concourse.bass import ts
    from concourse.masks import make_identity
    from concourse.tile import TileContext

    f32 = mybir.dt.float32
    bf16 = mybir.dt.bfloat16
    Exp = mybir.ActivationFunctionType.Exp

    nc = bass.Bass()
    x_d = nc.dram_tensor("x", [BPC, S, D], f32, kind="ExternalInput")
    wq_d = nc.dram_tensor("wq", [D, E], f32, kind="ExternalInput")
    wk_d = nc.dram_tensor("wk", [D, E], f32, kind="ExternalInput")
    wv_d = nc.dram_tensor("wv", [D, E], f32, kind="ExternalInput")
    out_d = nc.dram_tensor("out", [BPC, S, E], f32, kind="ExternalOutput")

    with TileContext(nc) as tc:
        with (
            tc.tile_pool(name="consts", bufs=1) as cpool,
            tc.tile_pool(name="xnat", bufs=3) as xpool,
            tc.tile_pool(name="xt", bufs=2) as xtpool,
            tc.tile_pool(name="qt", bufs=2) as qtpool,
            tc.tile_pool(name="kt", bufs=2) as ktpool,
            tc.tile_pool(name="vaug", bufs=2) as vpool,
            tc.tile_pool(name="pt", bufs=3) as ptpool,
            tc.tile_pool(name="ott", bufs=2) as otpool,
            tc.tile_pool(name="otile", bufs=4) as opool,
            tc.tile_pool(name="ps", bufs=2, space="PSUM") as pspool,
            tc.tile_pool(name="pss", bufs=2, space="PSUM") as pshalf,
            tc.tile_pool(name="acc", bufs=1, space="PSUM") as accpool,
        ):
            identb = cpool.tile([128, 128], bf16, tag="identb")
            make_identity(nc, identb[:])
            ident = cpool.tile([128, 128], f32, tag="ident")
            make_identity(nc, ident[:])

            # Wq/Wk packed side-by-side per d-chunk: chunk c occupies cols
            # [128c, 128c+64) = Wq, [128c+64, 128c+128) = Wk.  One [128,128]
            # stationary then projects Q^T and K^T in a single matmul stream.
            wstg = cpool.tile([128, 2 * E], f32, tag="wstg")
            wqk_sb = cpool.tile([128, 3 * 128], bf16, tag="wqk")
            wv_sb = cpool.tile([128, 3 * E], bf16, tag="wv")
            for c in range(3):
                nc.sync.dma_start(wstg[:, 0:E], wq_d[ts(c, 128), :])
                nc.sync.dma_start(wstg[:, E : 2 * E], wk_d[ts(c, 128), :])
                nc.vector.tensor_copy(
                    wqk_sb[:, 128 * c : 128 * c + 128], wstg[:]
                )
            for c in range(3):
                nc.sync.dma_start(wstg[:, 0:E], wv_d[ts(c, 128), :])
                nc.vector.tensor_copy(wv_sb[:, ts(c, E)], wstg[:, 0:E])

            H = 1024  # attention column-panel width
            NH = S // H

            def pieces512(lo, hi):
                out = []
                a = lo
                while a < hi:
                    b_ = min((a // 512 + 1) * 512, hi)
                    out.append((a, b_))
                    a = b_
                return out

            state = {}

            def phase_a(b):
                """x load/cast/transpose, QK projection, V+ones. Yields
                after each PSUM-consuming step so it can be interleaved
                into the previous batch's attention emission."""
                st = state[b] = {}
                xt_all = xtpool.tile([128, 3 * S], bf16, tag="xt", name=f"xt_{b}")
                xt3 = xt_all[:].rearrange("p (c s) -> p c s", c=3)
                for t in range(NB):
                    xn = xpool.tile([128, D], f32, tag="xn", name=f"xn_{b}_{t}")
                    nc.sync.dma_start(xn[:], x_d[b, ts(t, 128), :])
                    xb = xpool.tile([128, D], bf16, tag="xb", name=f"xb_{b}_{t}")
                    nc.vector.tensor_copy(xb[:], xn[:])
                    pxt = pspool.tile([128, 512], bf16, tag="ps", name=f"pxt_{b}_{t}")
                    for c in range(3):
                        nc.tensor.transpose(
                            pxt[:, 128 * c : 128 * c + 128],
                            xb[:, ts(c, 128)],
                            identb[:],
                        )
                    nc.vector.tensor_copy(
                        xt3[:, :, ts(t, 128)],
                        pxt[:, 0:D].rearrange("p (c s) -> p c s", c=3),
                    )
                    yield

                def xts(c, lo, width):
                    return xt_all[:, c * S + lo : c * S + lo + width]

                st["xts"] = xts
                qt = st["qt"] = qtpool.tile([64, S], bf16, tag="qt", name=f"qt_{b}")
                kt = st["kt"] = ktpool.tile([64, S], bf16, tag="kt", name=f"kt_{b}")
                for n in range(NCH):
                    pq = pspool.tile([128, 512], f32, tag="ps", name=f"pq_{b}_{n}")
                    for c in range(3):
                        nc.tensor.matmul(
                            pq[:],
                            wqk_sb[:, ts(c, 128)],
                            xts(c, 512 * n, 512),
                            start=(c == 0),
                            stop=(c == 2),
                        )
                    nc.vector.tensor_copy(qt[:, ts(n, 512)], pq[:64, :])
                    nc.vector.tensor_copy(kt[:, ts(n, 512)], pq[64:128, :])
                    yield

                va_all = st["va"] = vpool.tile(
                    [128, NB * (E + 1)], bf16, tag="va", name=f"va_{b}"
                )
                va3 = va_all[:].rearrange("p (k e) -> p k e", k=NB)
                for g in range(NB // 4):
                    pv = pspool.tile([128, 512], f32, tag="ps", name=f"pv_{b}_{g}")
                    for j in range(4):
                        k = 4 * g + j
                        for c in range(3):
                            nc.tensor.matmul(
                                pv[:, 64 * j : 64 * j + 64],
                                xts(c, 128 * k, 128),
                                wv_sb[:, ts(c, E)],
                                start=(c == 0),
                                stop=(c == 2),
                            )
                    nc.vector.tensor_copy(
                        va3[:, 4 * g : 4 * g + 4, 0:E],
                        pv[:, 0:256].rearrange("p (k e) -> p k e", k=4),
                    )
                    yield
                nc.gpsimd.memset(va3[:, :, E : E + 1], 1.0)

            def attention(b):
                """Panel attention + epilogue; yields per inner iteration."""
                st = state[b]
                qt, kt, va_all = st["qt"], st["kt"], st["va"]
                for h in range(NH):
                    base = H * h
                    acc = accpool.tile([E + 1, H], f32, tag="acc")
                    nblk = base // 128 + 8

                    def emit_pv(i, pt, qlo, acc=acc, base=base):
                        for (a, b_) in pieces512(qlo, base + H):
                            nc.tensor.matmul(
                                acc[:, a - base : b_ - base],
                                va_all[:, 65 * i : 65 * i + 65],
                                pt[:, a - qlo : b_ - qlo],
                                start=(i == 0),
                                stop=(i == (b_ - 1) // 128),
                            )

                    pending = None
                    for i in range(nblk):
                        qlo = max(128 * i, base)
                        w = base + H - qlo
                        ps_s = pshalf.tile([128, H], f32, tag="pss")
                        for c0 in range(0, w, 512):
                            c1 = min(c0 + 512, w)
                            nc.tensor.matmul(
                                ps_s[:, c0:c1],
                                kt[:, ts(i, 128)],
                                qt[:, qlo + c0 : qlo + c1],
                                start=True,
                                stop=True,
                            )
                        pt = ptpool.tile([128, H], bf16, tag="pt")
                        nc.scalar.activation(pt[:, :w], ps_s[:, :w], Exp, scale=SCALE)
                        if qlo == 128 * i:
                            nc.gpsimd.affine_select(
                                out=pt[:, 0:128],
                                in_=pt[:, 0:128],
                                compare_op=mybir.AluOpType.is_ge,
                                fill=0.0,
                                base=0,
                                pattern=[[1, 128]],
                                channel_multiplier=-1,
                            )
                        if pending is not None:
                            emit_pv(*pending)
                        pending = (i, pt, qlo)
                        yield
                    emit_pv(*pending)

                    ott = otpool.tile([E + 1, H], f32, tag="ott")
                    nc.vector.tensor_copy(ott[:], acc[:])
                    for tt in range(H // 128):
                        tg = (base // 128) + tt
                        pe_ = pspool.tile([128, 512], f32, tag="ps")
                        nc.tensor.transpose(
                            pe_[:, : E + 1],
                            ott[:, ts(tt, 128)],
                            ident[: E + 1, : E + 1],
                        )
                        rc = opool.tile([128, 1], f32, tag="rc")
                        nc.vector.reciprocal(rc[:], pe_[:, E : E + 1])
                        ot = opool.tile([128, E], f32, tag="ot")
                        nc.vector.tensor_scalar_mul(ot[:], pe_[:, 0:E], rc[:])
                        nc.sync.dma_start(out_d[b, ts(tg, 128), :], ot[:])
                        yield

            # Interleaved emission: batch b's attention iterations are
            # alternated with batch b+1's phase-A steps, so the in-order PE
            # queue always has transpose/projection matmuls to chew on while
            # ACT computes exp() for the current attention block.
            for _ in phase_a(0):
                pass
            pa_next = phase_a(1)
            for _ in attention(0):
                next(pa_next, None)
            for _ in pa_next:
                pass
            for _ in attention(1):
                pass

    _split_multi_waits(nc)
    return nc


def _get_nc():
    if "nc" not in _cache:
        _install_patches()
        _cache["nc"] = _build_nc()
    return _cache["nc"]


def kernel(x, Wq, Wk, Wv):
    from concourse.bass_utils import run_bass_kernel_spmd

    nc = _get_nc()
    x = np.ascontiguousarray(x, dtype=np.float32)
    in_maps = [
        {
            "x": x[i * BPC : (i + 1) * BPC],
            "wq": np.asarray(Wq, dtype=np.float32),
            "wk": np.asarray(Wk, dtype=np.float32),
            "wv": np.asarray(Wv, dtype=np.float32),
        }
        for i in range(N_CORES)
    ]
    res = run_bass_kernel_spmd(nc, in_maps, list(range(N_CORES)))
    out = np.concatenate([res.results[i]["out"] for i in range(N_CORES)], axis=0)
    return out.astype(np.float32)
